# revision 32
# baseline (speedup 1.0000x reference)
"""CRPS loss kernel for Trainium2, data-parallel over 8 NeuronCores.

Math (per sample n, timestep t, quantiles q_0..q_10 sorted, target y):
  O_i = [q_i >= y], u_i = Gt_i*O_i with Gt = 100*(1 - i/5)
  w'_1..9 = u_{i-1} - u_{i+1} + Bt_i      (interior trapz weights)
  w'_0 = u_0 - u_1 + Bt_0 ; w'_10 = u_9 - u_10 + Bt_10   (relu q-parts folded)
  trapz = sum_i q_i*w'_i / 200 + t*(1 - O_0 - O_10)
  out[n] = mean_t trapz
All constants are integers <= 256 after the x100 scaling, so the bf16
pipeline (O, u, w') is exact; reductions accumulate in f32.

Layout: each core gets 512 samples; 2 super-blocks x (2 sample-segments x
128 partitions); partition = sample, free = (segment*t, quantile).
Elementwise ops run merged over both segments (FD=11264); the per-sample
fused multiply+accumulate reductions run per segment.
"""
import sys

if "/opt/trn_rl_repo" not in sys.path:
    sys.path.insert(0, "/opt/trn_rl_repo")

import numpy as np
import concourse.bass as bass
import concourse.tile as tile
from concourse import bacc, mybir
from concourse.bass_utils import run_bass_kernel_spmd
from concourse.alu_op_type import AluOpType

N_CORES = 8
N, T, D = 4096, 512, 11
N_LOC = N // N_CORES        # 512 samples per core
P = 128                     # partitions
SEG = 2                     # sample-segments per super-block
SB = N_LOC // (P * SEG)     # 2 super-blocks
T2 = SEG * T                # merged free rows per partition
FP = mybir.dt.float32
BF = mybir.dt.bfloat16

# quantile-axis constants scaled by LAM=100 -> small integers, bf16-exact
LAM = 100.0
_F = np.arange(D) / 10.0
_A = _F * _F
_G = 1.0 - 2.0 * _F
_BETA = np.empty(D)
_BETA[0] = -(_A[0] + _A[1])
for _j in range(1, D - 1):
    _BETA[_j] = _A[_j - 1] - _A[_j + 1]
_BETA[D - 1] = _A[D - 2] + _A[D - 1] - 2.0
_GT = np.rint(LAM * _G)      # [100, 80, ..., -100]
_BT = np.rint(LAM * _BETA)   # [-1, -4, -8, ..., -36, -19]

N_DMA_SPLIT = 4  # chunks per sample-segment load


def build_consts(tc, const_pool):
    """Materialize full-width bf16 constant tiles (one-time)."""
    nc = tc.nc
    g_s = const_pool.tile([P, D], BF, tag="gs")
    b_s = const_pool.tile([P, D], BF, tag="bs")
    for i in range(D):
        nc.vector.memset(g_s[:, i : i + 1], float(_GT[i]))
        nc.vector.memset(b_s[:, i : i + 1], float(_BT[i]))
    g_f = const_pool.tile([P, T2 * D], BF, tag="gf")
    b_f = const_pool.tile([P, T2 * D], BF, tag="bf")
    g_f3 = g_f[:].rearrange("p (t i) -> p t i", i=D)
    b_f3 = b_f[:].rearrange("p (t i) -> p t i", i=D)
    nc.vector.tensor_copy(g_f3, g_s[:].unsqueeze(1).broadcast_to([P, T2, D]))
    nc.vector.tensor_copy(b_f3, b_s[:].unsqueeze(1).broadcast_to([P, T2, D]))
    return g_f, b_f


def build_crps_kernel(tc, out_ap, inp_ap, tgt_ap, g_f, b_f):
    nc = tc.nc

    # sample n = b*256 + s*128 + p  ->  accumulator column k = 2*b + s
    inp_r = inp_ap.rearrange("(b s p) t i -> b s p (t i)", s=SEG, p=P)
    tgt_r = tgt_ap.rearrange("(b s p) t -> b s p t", s=SEG, p=P)

    with (
        tc.tile_pool(name="data", bufs=2) as data_pool,
        tc.tile_pool(name="work", bufs=1) as work_pool,
        tc.tile_pool(name="acc", bufs=2) as acc_pool,
    ):
        ncol = SB * SEG
        s_w4 = acc_pool.tile([P, ncol], FP, tag="sw4")
        st1 = acc_pool.tile([P, ncol], FP, tag="st1")
        st0 = acc_pool.tile([P, ncol], FP, tag="st0")
        for b in range(SB):
            q = data_pool.tile([P, T2 * D], FP, tag="q")
            chunk = (T * D) // N_DMA_SPLIT
            for s in range(SEG):
                for c in range(N_DMA_SPLIT):
                    nc.sync.dma_start(
                        q[:, s * T * D + c * chunk : s * T * D + (c + 1) * chunk],
                        inp_r[b, s][:, c * chunk : (c + 1) * chunk],
                    )
            tg = work_pool.tile([P, T2], FP, tag="tg")
            for s in range(SEG):
                nc.sync.dma_start(tg[:, s * T : (s + 1) * T], tgt_r[b, s])

            q3 = q[:].rearrange("p (t i) -> p t i", i=D)
            tgB = tg[:].unsqueeze(2).broadcast_to([P, T2, D])

            # O = [q >= y]  (exact f32 compare, bf16 0/1 output)
            o = work_pool.tile([P, T2 * D], BF, tag="o")
            o3 = o[:].rearrange("p (t i) -> p t i", i=D)
            nc.vector.tensor_tensor(o3, q3, tgB, AluOpType.is_ge)

            # u = Gt * O   (bf16 2x, exact)
            u = work_pool.tile([P, T2 * D], BF, tag="u")
            u3 = u[:].rearrange("p (t i) -> p t i", i=D)
            nc.vector.tensor_tensor(u[:], o[:], g_f[:], AluOpType.mult)

            # w built in place of o (o is dead after u); integer diffs exact
            w3 = o3
            uT = u[:].rearrange("p (t i) -> p i t", i=D)
            wT = o[:].rearrange("p (t i) -> p i t", i=D)
            nc.vector.tensor_tensor(
                w3[:, :, 1:10], u3[:, :, 0:9], u3[:, :, 2:11], AluOpType.subtract
            )
            # w'_0 = u_0 - u_1 ; w'_10 = u_9 - u_10 (relu q-parts folded in)
            nc.vector.scalar_tensor_tensor(
                wT[:, 0:1, :], uT[:, 0:1, :], 1.0, uT[:, 1:2, :],
                AluOpType.mult, AluOpType.subtract,
            )
            nc.vector.tensor_tensor(
                wT[:, 10:11, :], uT[:, 9:10, :], uT[:, 10:11, :],
                AluOpType.subtract,
            )
            # d = u_0 - u_10 = 100*(O_0 + O_10) for the t-branch
            dmm = work_pool.tile([P, T2], FP, tag="dmm")
            nc.vector.tensor_tensor(
                dmm[:].unsqueeze(1), uT[:, 0:1, :], uT[:, 10:11, :],
                AluOpType.subtract,
            )

            # w' = w + Bt  (bf16 2x, exact) -> into u tile
            nc.vector.tensor_tensor(u[:], o[:], b_f[:], AluOpType.add)

            # per-sample-segment reductions (accumulate in f32); the stt
            # scratch output goes into the dead o tile
            for s in range(SEG):
                k = SEG * b + s
                lo, hi = s * T * D, (s + 1) * T * D
                q3s = q[:, lo:hi].rearrange("p (t i) -> p t i", i=D)
                w3s = u[:, lo:hi].rearrange("p (t i) -> p t i", i=D)
                o3s = o[:, lo:hi].rearrange("p (t i) -> p t i", i=D)
                # S = sum_{t,i} q * w'
                nc.vector.scalar_tensor_tensor(
                    o3s, q3s, 1.0, w3s, AluOpType.mult, AluOpType.mult,
                    accum_out=s_w4[:, k : k + 1],
                )
                tgs = tg[:, s * T : (s + 1) * T]
                dms = dmm[:, s * T : (s + 1) * T]
                # ST1 = sum_t t*100*(O_0+O_10)
                nc.vector.scalar_tensor_tensor(
                    o[:, lo : lo + T], tgs, 1.0, dms,
                    AluOpType.mult, AluOpType.mult,
                    accum_out=st1[:, k : k + 1],
                )
                # ST0 = sum_t t
                nc.vector.tensor_scalar(
                    o[:, lo + T : lo + 2 * T], tgs, 1.0, 0.0, AluOpType.mult,
                    AluOpType.add, accum_out=st0[:, k : k + 1],
                )

        # r = (S/(2*LAM) + ST0 - ST1/LAM) / T for all sample groups at once
        r1 = acc_pool.tile([P, ncol], FP, tag="r1")
        nc.vector.scalar_tensor_tensor(
            r1[:], st1[:], -1.0 / LAM, st0[:], AluOpType.mult, AluOpType.add,
        )
        r2 = acc_pool.tile([P, ncol], FP, tag="r2")
        nc.vector.scalar_tensor_tensor(
            r2[:], s_w4[:], 0.5 / LAM, r1[:], AluOpType.mult, AluOpType.add,
        )
        r3 = acc_pool.tile([P, ncol], FP, tag="r3")
        nc.vector.tensor_scalar_mul(r3[:], r2[:], 1.0 / T)
        # out[(k p)]: partition p writes ncol floats at stride P
        nc.sync.dma_start(out_ap.rearrange("(k p) -> p k", p=P), r3[:])


def _build_nc(repeat=1):
    nc = bacc.Bacc("TRN2", target_bir_lowering=False, debug=False,
                   num_devices=N_CORES)
    inp = nc.dram_tensor("inp", [N_LOC, T, D], FP, kind="ExternalInput").ap()
    tgt = nc.dram_tensor("target", [N_LOC, T], FP, kind="ExternalInput").ap()
    out = nc.dram_tensor("out", [N_LOC], FP, kind="ExternalOutput").ap()
    with tile.TileContext(nc) as tc:
        with tc.tile_pool(name="const", bufs=1) as const_pool:
            g_f, b_f = build_consts(tc, const_pool)
            if repeat == 1:
                build_crps_kernel(tc, out, inp, tgt, g_f, b_f)
            else:
                with tc.For_i(0, repeat, 1):
                    build_crps_kernel(tc, out, inp, tgt, g_f, b_f)
    nc.compile()
    return nc


_NC_CACHE = {}


def get_nc(repeat=1):
    if repeat not in _NC_CACHE:
        _NC_CACHE[repeat] = _build_nc(repeat)
    return _NC_CACHE[repeat]


def kernel(inp: np.ndarray, target: np.ndarray) -> np.ndarray:
    inp = np.ascontiguousarray(inp, dtype=np.float32)
    target = np.ascontiguousarray(target, dtype=np.float32)
    nc = get_nc()
    in_maps = [
        {
            "inp": inp[c * N_LOC : (c + 1) * N_LOC],
            "target": target[c * N_LOC : (c + 1) * N_LOC],
        }
        for c in range(N_CORES)
    ]
    res = run_bass_kernel_spmd(nc, in_maps, core_ids=list(range(N_CORES)))
    return np.concatenate([res.results[c]["out"] for c in range(N_CORES)])
